# revision 24
# baseline (speedup 1.0000x reference)
"""Multi-head attention Trainium2 Bass kernel (v2).

Problem: B=8, S=1024, E=768, H=12, DH=64 MHA with per-head Q/K/V projections
and output projection. Data-parallel over batch: one batch element per
NeuronCore (8 cores).

Per-core dataflow (attention matmul flipped so Z lands on partitions,
normalization is a per-partition DVE multiply, and the output projection
keeps Wo stationary writing a transposed output):
  xT [E,S] bf16  <- DMA-transpose of x
  qT/kT = W.T @ xT + b per head-pair [128(d-pair), S] bf16
  v [s, d-pair] produced directly (xT chunk stationary, Wv moving) into
     vOnes with ones columns; bias via host-replicated [128,128] table
  scoresT[t,s] = k @ q.T per head, row-packed K=64 matmuls
  expST = exp(0.125*scoresT) (ACT, bf16; no max subtraction: scores ~ N(0,1))
  attn[sq, d|Z] = expST_chunk.T @ [v|1]  (ex stationary [t,sq], M=128, N=65;
     Z accumulates in column 64, so the row sum costs one extra column)
  a_sb = attn * (1/Z)  (DVE reciprocal + free-dim-broadcast tensor_tensor)
  catT[d-pair, s] <- PE transpose of a_sb in [128,128] blocks
  outT[eo, s] = Wo_chunk.T @ catT + bo (Wo stationary; per-partition DVE
     bias add on the PSUM drain; host transposes the [E,S] result)
"""
import sys

sys.path.insert(0, "/opt/trn_rl_repo")

import numpy as np
import ml_dtypes
from contextlib import ExitStack

import concourse.bass as bass
import concourse.tile as tile
from concourse import bacc, mybir
from concourse.bass_utils import run_bass_kernel_spmd
from concourse.masks import make_identity

F32 = mybir.dt.float32
BF16 = mybir.dt.bfloat16
AF = mybir.ActivationFunctionType
BF = ml_dtypes.bfloat16

B, S, E, H, DH = 8, 1024, 768, 12, 64
NP_ = 6          # head pairs
ET = 6           # e tiles of 128
ST = 8           # s tiles of 128
NCORES = 8

_cache = {}


def _build_nc(reps=1, ablate=""):
    if ("nc", reps, ablate) in _cache:
        return _cache[("nc", reps, ablate)]
    nc = bacc.Bacc("TRN2", target_bir_lowering=False, debug=False,
                   num_devices=NCORES)

    x = nc.dram_tensor("x", [S, E], BF16, kind="ExternalInput").ap()
    wq = nc.dram_tensor("wq", [NP_, 128, ET, 128], BF16, kind="ExternalInput").ap()
    wk = nc.dram_tensor("wk", [NP_, 128, ET, 128], BF16, kind="ExternalInput").ap()
    wv = nc.dram_tensor("wv", [NP_, 128, ET, 128], BF16, kind="ExternalInput").ap()
    bq = nc.dram_tensor("bq", [NP_, 128, 1], F32, kind="ExternalInput").ap()
    bk = nc.dram_tensor("bk", [NP_, 128, 1], F32, kind="ExternalInput").ap()
    bvr = nc.dram_tensor("bvr", [128, NP_, 128], F32, kind="ExternalInput").ap()
    wo = nc.dram_tensor("wo", [128, ET * E], BF16, kind="ExternalInput").ap()
    boT = nc.dram_tensor("boT", [128, ET], F32, kind="ExternalInput").ap()
    out = nc.dram_tensor("out", [E, S], F32, kind="ExternalOutput").ap()

    oldattn = "oldattn" in ablate

    with tile.TileContext(nc) as tc, ExitStack() as ctx:
        consts = ctx.enter_context(tc.tile_pool(name="consts", bufs=1))
        xtp = ctx.enter_context(tc.tile_pool(
            name="xtp", bufs=2 if "xtp2" in ablate else 1))
        catp = ctx.enter_context(tc.tile_pool(name="catp", bufs=1))
        wpool = ctx.enter_context(tc.tile_pool(name="wpool", bufs=3))
        qkp = ctx.enter_context(tc.tile_pool(name="qkp", bufs=3))
        vop = ctx.enter_context(tc.tile_pool(name="vop", bufs=3))
        exq = ctx.enter_context(tc.tile_pool(name="exq", bufs=3))
        asb = ctx.enter_context(tc.tile_pool(name="asb", bufs=2))
        zrp = ctx.enter_context(tc.tile_pool(name="zrp", bufs=4))
        osb = ctx.enter_context(tc.tile_pool(name="osb", bufs=3))
        if oldattn:
            zp = ctx.enter_context(tc.tile_pool(name="zp", bufs=2))
            cup = ctx.enter_context(tc.tile_pool(name="cup", bufs=3))
            zdp = ctx.enter_context(tc.tile_pool(name="zdp", bufs=4,
                                                 space="DRAM"))
        # PSUM: 2 + 4 + 2 = 8 banks (scp3: 2 + 3 + 3)
        scp3 = "scp3" in ablate
        mmp = ctx.enter_context(tc.tile_pool(name="mmp", bufs=2, space="PSUM"))
        scp = ctx.enter_context(tc.tile_pool(
            name="scp", bufs=3 if scp3 else 2, space="PSUM"))
        atp = ctx.enter_context(tc.tile_pool(
            name="atp", bufs=3 if scp3 else 2, space="PSUM"))

        ident = consts.tile([128, 128], BF16, tag="ident")
        make_identity(nc, ident)
        boT_t = consts.tile([128, ET], F32, tag="boT")
        nc.sync.dma_start(boT_t, boT)
        wo_t = consts.tile([128, ET * E], BF16, tag="wo")
        nc.sync.dma_start(wo_t, wo)
        bvr_t = consts.tile([128, NP_, 128], F32, tag="bvr")
        nc.sync.dma_start(bvr_t, bvr)

        for _rep in range(reps):
            # ---- Phase 0: xT [E, S] via DMA transpose ----
            xT = [xtp.tile([128, S], BF16, tag=f"xT{et}", name=f"xT{et}")
                  for et in range(ET)]
            for et in range(ET):
                nc.sync.dma_start(
                    xT[et], x[:, et * 128:(et + 1) * 128], transpose=True)

            catT = [catp.tile([128, S], BF16, tag=f"catT{j}", name=f"catT{j}")
                    for j in range(NP_)]

            # ---- Per head-pair, software-pipelined: pair p's attention
            # consumption (attn matmuls + normalize + transpose back) is
            # emitted after pair p+1's production (QKV + scores + exp) so
            # the PE always has ready work while ACT runs the exps. ----
            def produce(p, cons_iter=None):
                wq_t = wpool.tile([128, ET, 128], BF16, tag="wq", name="wq_t")
                nc.sync.dma_start(wq_t, wq[p])
                wk_t = wpool.tile([128, ET, 128], BF16, tag="wk", name="wk_t")
                nc.sync.dma_start(wk_t, wk[p])
                wv_t = wpool.tile([128, ET, 128], BF16, tag="wv", name="wv_t")
                nc.sync.dma_start(wv_t, wv[p])
                bq_t = wpool.tile([128, 1], F32, tag="bq", name="bq_t")
                nc.sync.dma_start(bq_t, bq[p])
                bk_t = wpool.tile([128, 1], F32, tag="bk", name="bk_t")
                nc.sync.dma_start(bk_t, bk[p])

                qT = qkp.tile([128, S], BF16, tag="qT", name="qT")
                kT = qkp.tile([128, S], BF16, tag="kT", name="kT")
                for w_t, b_t, dst in ((wq_t, bq_t, qT), (wk_t, bk_t, kT)):
                    for ch in range(2):
                        pp = mmp.tile([128, 512], F32, tag="mm", name="pp")
                        for et in range(ET):
                            nc.tensor.matmul(
                                pp, w_t[:, et, :],
                                xT[et][:, ch * 512:(ch + 1) * 512],
                                start=(et == 0), stop=(et == ET - 1),
                            )
                        nc.vector.tensor_scalar_add(
                            dst[:, ch * 512:(ch + 1) * 512], pp, b_t)

                # v directly in [s, d-pair] layout (xT chunk stationary,
                # Wv moving), packed into vOnes with ones columns; bias
                # comes from a host-replicated [128, 128] table.
                vo = vop.tile([128, ST, 130], BF16, tag="vo", name="vo")
                nc.gpsimd.memset(vo.rearrange("p t d -> p (t d)"), 1.0)
                bvp = bvr_t[:, p, :].rearrange("p (two d) -> p two d", two=2)
                for stp in range(4):
                    pv = mmp.tile([128, 2, 128], F32, tag="mm", name="pv")
                    for s2 in range(2):
                        st = stp * 2 + s2
                        for et in range(ET):
                            nc.tensor.matmul(
                                pv[:, s2, :],
                                xT[et][:, st * 128:(st + 1) * 128],
                                wv_t[:, et, :],
                                start=(et == 0), stop=(et == ET - 1),
                            )
                    for s2 in range(2):
                        st = stp * 2 + s2
                        nc.vector.tensor_tensor(
                            out=vo[:, st, :].rearrange(
                                "p (two dd) -> p two dd", two=2)[:, :, 0:64],
                            in0=pv[:, s2, :].rearrange(
                                "p (two d) -> p two d", two=2),
                            in1=bvp,
                            op=mybir.AluOpType.add,
                        )

                # scores for both heads interleaved (K=64 row-packing).
                # Units of the previous pair's attention are emitted between
                # score tiles so the in-order PE fills its ACT-wait stalls.
                ex_ts = [exq.tile([128, ST, S], BF16, tag=f"ex{e}",
                                  name=f"ex{e}") for e in range(2)]
                cnt = 0
                for t in range(ST):
                    for e in range(2):
                        r0 = 64 * e
                        if scp3:
                            for ch in range(2):
                                sc = scp.tile([128, 512], F32, tag="sc",
                                              name="sc")
                                nc.tensor.matmul(
                                    sc,
                                    kT[r0:r0 + 64, t * 128:(t + 1) * 128],
                                    qT[r0:r0 + 64, ch * 512:(ch + 1) * 512],
                                    tile_position=(r0, 0),
                                    start=True, stop=True,
                                    skip_group_check=True,
                                )
                                nc.scalar.activation(
                                    ex_ts[e][:, t, ch * 512:(ch + 1) * 512],
                                    sc, AF.Exp, scale=0.125)
                        else:
                            sc = scp.tile([128, S], F32, tag="sc", name="sc")
                            for ch in range(2):
                                nc.tensor.matmul(
                                    sc[:, ch * 512:(ch + 1) * 512],
                                    kT[r0:r0 + 64, t * 128:(t + 1) * 128],
                                    qT[r0:r0 + 64, ch * 512:(ch + 1) * 512],
                                    tile_position=(r0, 0),
                                    start=True, stop=True,
                                    skip_group_check=True,
                                )
                            nc.scalar.activation(ex_ts[e][:, t, :], sc,
                                                 AF.Exp, scale=0.125)
                        cnt += 1
                        if cons_iter is not None and cnt % 3 == 0:
                            next(cons_iter, None)
                return (vo, ex_ts)

            def consume_gen(p, state):
                vo, ex_ts = state
                # Flipped attention: ex chunk stationary [128(t), 128(sq)],
                # [v|1] moving (N=65). Z lands in column 64 on the sq
                # partition, so normalization is a per-partition DVE
                # tensor_scalar_mul on the PSUM drain. Both heads land in
                # one a_sb [sq, st, d-pair] so the transpose back to
                # catT [d-pair, s] runs as full [128,128] blocks.
                a_sb = asb.tile([128, ST, 128], BF16, tag="asb", name="asb")
                for e in range(2):
                    ex_t = ex_ts[e]
                    for sh in range(2):
                        ap_ = atp.tile([128, 4, 65], F32, tag="att",
                                       name="att")
                        for sq4 in range(4):
                            sq = sh * 4 + sq4
                            for t in range(ST):
                                nc.tensor.matmul(
                                    ap_[:, sq4, :],
                                    ex_t[:, t, sq * 128:(sq + 1) * 128],
                                    vo[:, t, 65 * e:65 * e + 65],
                                    start=(t == 0), stop=(t == ST - 1),
                                )
                        with tc.high_priority(offset=150):
                            zr = zrp.tile([128, 4], F32, tag="zr", name="zr")
                            nc.vector.reciprocal(zr, ap_[:, :, 64])
                            nc.vector.tensor_tensor(
                                out=a_sb[:, sh * 4:(sh + 1) * 4,
                                         64 * e:64 * e + 64],
                                in0=ap_[:, :, 0:64],
                                in1=zr[:, :, None].broadcast_to([128, 4, 64]),
                                op=mybir.AluOpType.mult,
                            )
                        yield
                # transpose a_sb [sq, d-pair] -> catT [d-pair, s]
                for g in range(2):
                    tp = atp.tile([128, 4, 128], BF16, tag="att", name="tpa")
                    for k in range(4):
                        st = g * 4 + k
                        nc.tensor.matmul(
                            tp[:, k, :], a_sb[:, st, :], ident,
                            is_transpose=True, skip_group_check=True,
                            start=True, stop=True,
                        )
                    cp_eng = (nc.scalar.copy if "cpact" in ablate
                              else nc.vector.tensor_copy)
                    cp_eng(
                        catT[p][:, g * 512:(g + 1) * 512],
                        tp.rearrange("p t d -> p (t d)"))
                    yield

            def consume_old(p, state):
                vo, ex_ts = state
                catU = cup.tile([128, S], BF16, tag="catU", name="catU")
                deferred = []
                for e in range(2):
                    r0 = 64 * e
                    ex_t = ex_ts[e]
                    for ch in range(2):
                        ap_ = atp.tile([65, 512], F32, tag="att", name="att")
                        for t in range(ST):
                            nc.tensor.matmul(
                                ap_, vo[:, t, 65 * e:65 * e + 65],
                                ex_t[:, t, ch * 512:(ch + 1) * 512],
                                start=(t == 0), stop=(t == ST - 1),
                            )
                        zrec = zp.tile([1, 512], F32, tag="zrec", name="zrec",
                                       bufs=4)
                        with tc.high_priority(offset=150):
                            nc.vector.reciprocal(zrec, ap_[64:65, :])
                            nc.vector.tensor_copy(
                                catU[r0:r0 + 64, ch * 512:(ch + 1) * 512],
                                ap_[0:64, :])
                        zd = zdp.tile([1, 512], F32, tag="zd", name="zd")
                        nc.sync.dma_start(zd, zrec)
                        zrep = zp.tile([128, 512], F32, tag="zrep",
                                       name="zrep", bufs=4)
                        nc.sync.dma_start(zrep, zd.partition_broadcast(128))
                        deferred.append((r0, ch, zrep))
                for r0, ch, zrep in deferred:
                    nc.gpsimd.tensor_tensor(
                        out=catT[p][r0:r0 + 64, ch * 512:(ch + 1) * 512],
                        in0=catU[r0:r0 + 64, ch * 512:(ch + 1) * 512],
                        in1=zrep[r0:r0 + 64, :],
                        op=mybir.AluOpType.mult,
                    )

            if oldattn:
                states = {}
                for p in range(NP_):
                    states[p] = produce(p)
                    if p >= 2:
                        consume_old(p - 2, states.pop(p - 2))
                consume_old(NP_ - 2, states.pop(NP_ - 2))
                consume_old(NP_ - 1, states.pop(NP_ - 1))
            else:
                states = {}
                for p in range(NP_):
                    ci = None
                    if p >= 2:
                        ci = consume_gen(p - 2, states.pop(p - 2))
                    states[p] = produce(p, ci)
                    if ci is not None:
                        for _ in ci:
                            pass
                for q in (NP_ - 2, NP_ - 1):
                    for _ in consume_gen(q, states.pop(q)):
                        pass

            # ---- Output projection (Wo stationary, transposed output) ----
            # Alternate PSUM between the mm and sc tags so consecutive eo
            # groups never wait on each other's drains.
            for eo in range(ET):
                pool, ptag = (mmp, "mm") if eo % 2 == 0 else (scp, "sc")
                pps = [pool.tile([128, 512], F32, tag=ptag, name=f"op{ch}")
                       for ch in range(2)]
                for j in range(NP_):
                    w_sl = wo_t[:, j * E + eo * 128:j * E + eo * 128 + 128]
                    for ch in range(2):
                        nc.tensor.matmul(
                            pps[ch], w_sl,
                            catT[j][:, ch * 512:(ch + 1) * 512],
                            start=(j == 0), stop=(j == NP_ - 1),
                        )
                for ch in range(2):
                    o_sb = osb.tile([128, 512], F32, tag="ot", name="ot")
                    if "obact" in ablate:
                        nc.scalar.activation(o_sb, pps[ch], AF.Identity,
                                             bias=boT_t[:, eo:eo + 1])
                    else:
                        nc.vector.tensor_scalar_add(o_sb, pps[ch],
                                                    boT_t[:, eo:eo + 1])
                    nc.sync.dma_start(
                        out[eo * 128:(eo + 1) * 128,
                            ch * 512:(ch + 1) * 512], o_sb)

    nc.compile()
    _cache[("nc", reps, ablate)] = nc
    return nc


def _prep_weights(Wq, bq, Wk, bk, Wv, bv, Wo, bo):
    def pack_w(W):  # [12, 768, 64] -> [6, 128, 6, 128] bf16
        Wp = W.reshape(NP_, 2, E, DH).transpose(0, 2, 1, 3).reshape(NP_, E, 128)
        return np.ascontiguousarray(
            Wp.reshape(NP_, ET, 128, 128).transpose(0, 2, 1, 3)).astype(BF)

    def pack_b(b):  # [12, 64] -> [6, 128, 1] f32
        return np.ascontiguousarray(b.reshape(NP_, 128, 1)).astype(np.float32)

    return {
        "wq": pack_w(Wq), "wk": pack_w(Wk), "wv": pack_w(Wv),
        "bq": pack_b(bq), "bk": pack_b(bk),
        "bvr": np.ascontiguousarray(np.broadcast_to(
            bv.reshape(1, NP_, 128), (128, NP_, 128))).astype(np.float32),
        "wo": np.ascontiguousarray(
            Wo.reshape(ET, 128, E).transpose(1, 0, 2).reshape(128, ET * E)
        ).astype(BF),
        "boT": np.ascontiguousarray(
            bo.reshape(ET, 128).T).astype(np.float32),
    }


def kernel(hidden_state, Wq, bq, Wk, bk, Wv, bv, Wo, bo):
    hidden_state = np.asarray(hidden_state, dtype=np.float32)
    shared = _prep_weights(
        np.asarray(Wq, np.float32), np.asarray(bq, np.float32),
        np.asarray(Wk, np.float32), np.asarray(bk, np.float32),
        np.asarray(Wv, np.float32), np.asarray(bv, np.float32),
        np.asarray(Wo, np.float32), np.asarray(bo, np.float32))
    nc = _build_nc()
    in_maps = [
        {"x": np.ascontiguousarray(hidden_state[b]).astype(BF), **shared}
        for b in range(NCORES)
    ]
    res = run_bass_kernel_spmd(nc, in_maps, core_ids=list(range(NCORES)))
    return np.stack([np.ascontiguousarray(r["out"].T) for r in res.results],
                    axis=0)
